# revision 1
# baseline (speedup 1.0000x reference)
"""Lookahead depthwise convolution on 8 Trainium2 NeuronCores.

out[t, b, f] = sum_{c=0..K-1} x[t+c, b, f] * weight[f, c], zero-padded at the
right edge. x: (2048, 32, 1280) fp32, weight: (1280, 81) fp32.

Strategy: shard the (fully independent) feature dim across 8 cores, 160
features each. Per feature the time conv is a banded Toeplitz matmul: with
128-wide time tiles, out_i = A_f @ x_i + B_f @ x_{i+1} where
  A_f[p, m] = w[f, p - m]        (0 <= p - m < K)
  B_f[p, m] = w[f, p + 128 - m]  (0 <= p + 128 - m < K)
Both 128x128 stationary matrices per feature are precomputed on the host in
fp16 and stay resident in SBUF. fp16 x fp16 products are exact in the fp32
PSUM accumulator, so the only error is fp16 input rounding (~1e-3 rel).
"""

import numpy as np

import concourse.bass as bass
import concourse.bacc as bacc
import concourse.mybir as mybir
from concourse import tile
from concourse.bass_utils import run_bass_kernel_spmd

S, B, F, K = 2048, 32, 1280, 81
N_CORES = 8
FC = F // N_CORES          # features per core (160)
TB = S // 128              # time blocks (16)
FPB = 16                   # features evicted per PSUM bank (16 * 32 = 512)
BAND_FREE = FC * 2 * 128   # free-dim elems of the resident band tile (40960)

_compiled = None


def _build_program():
    nc = bacc.Bacc("TRN2", target_bir_lowering=False, debug=False)
    f32, f16 = mybir.dt.float32, mybir.dt.float16

    x_in = nc.declare_dram_parameter("x", [S, B, FC], f32, isOutput=False)
    bands_in = nc.declare_dram_parameter("bands", [128, BAND_FREE], f16,
                                         isOutput=False)
    out_ext = nc.declare_dram_parameter("out", [S, B, FC], f32, isOutput=True)

    x_flat = x_in.rearrange("s b f -> s (b f)")
    out_flat = out_ext.rearrange("s b f -> s (b f)")
    BF = B * FC  # 5120

    with tile.TileContext(nc) as tc:
        with (
            tc.tile_pool(name="bands", bufs=1) as bpool,
            tc.tile_pool(name="x32", bufs=2) as x32pool,
            tc.tile_pool(name="x16", bufs=4) as x16pool,
            tc.tile_pool(name="stage", bufs=2) as spool,
            tc.tile_pool(name="psum", bufs=6, space="PSUM") as ppool,
        ):
            band_tile = bpool.tile([128, BAND_FREE], f16)
            nc.gpsimd.dma_start(out=band_tile[:], in_=bands_in[:])

            def load_block(i):
                x32 = x32pool.tile([128, BF], f32)
                nc.gpsimd.dma_start(
                    out=x32[:], in_=x_flat[i * 128:(i + 1) * 128, :])
                x16 = x16pool.tile([128, BF], f16)
                nc.scalar.copy(out=x16[:], in_=x32[:])
                return x16

            x16_cur = load_block(0)
            for i in range(TB):
                x16_nxt = load_block(i + 1) if i + 1 < TB else None
                stage = spool.tile([128, BF], f32)
                cur_v = x16_cur.rearrange("t (b f) -> t b f", f=FC)
                nxt_v = (x16_nxt.rearrange("t (b f) -> t b f", f=FC)
                         if x16_nxt is not None else None)
                for g in range(FC // FPB):          # 10 psum-bank groups
                    psum = ppool.tile([128, FPB * B], f32)
                    for j in range(FPB):
                        f = g * FPB + j
                        lA = band_tile[:, f * 256:f * 256 + 128]
                        nc.tensor.matmul(
                            out=psum[:, j * B:(j + 1) * B],
                            lhsT=lA, rhs=cur_v[:, :, f],
                            start=True, stop=(nxt_v is None))
                        if nxt_v is not None:
                            lB = band_tile[:, f * 256 + 128:f * 256 + 256]
                            nc.tensor.matmul(
                                out=psum[:, j * B:(j + 1) * B],
                                lhsT=lB, rhs=nxt_v[:, :, f],
                                start=False, stop=True)
                    # psum free layout is (j, b); stage wants (b, f)
                    psum_v = psum.rearrange("t (j b) -> t b j", j=FPB)
                    stage_v = stage.rearrange("t (b f) -> t b f", f=FC)
                    nc.vector.tensor_copy(
                        out=stage_v[:, :, g * FPB:(g + 1) * FPB],
                        in_=psum_v)
                nc.gpsimd.dma_start(
                    out=out_flat[i * 128:(i + 1) * 128, :], in_=stage[:])
                x16_cur = x16_nxt
    nc.finalize()
    return nc


def _build_bands(weight):
    p = np.arange(128)[:, None]
    m = np.arange(128)[None, :]
    dA = p - m
    dB = p + 128 - m
    mA = (dA >= 0) & (dA < K)
    mB = (dB >= 0) & (dB < K)
    iA = np.clip(dA, 0, K - 1)
    iB = np.clip(dB, 0, K - 1)
    w16 = weight.astype(np.float16).astype(np.float32)
    A = w16[:, iA] * mA          # [F, 128p, 128m]
    Bm = w16[:, iB] * mB
    bands = np.empty((128, F, 2, 128), np.float16)
    bands[:, :, 0, :] = A.transpose(1, 0, 2)
    bands[:, :, 1, :] = Bm.transpose(1, 0, 2)
    return bands


def kernel(x, weight):
    global _compiled
    x = np.asarray(x, dtype=np.float32)
    weight = np.asarray(weight, dtype=np.float32)
    if _compiled is None:
        _compiled = _build_program()
    nc = _compiled
    bands = _build_bands(weight)
    in_maps = []
    for c in range(N_CORES):
        fl = slice(c * FC, (c + 1) * FC)
        in_maps.append({
            "x": np.ascontiguousarray(x[:, :, fl]),
            "bands": np.ascontiguousarray(
                bands[:, fl, :, :]).reshape(128, BAND_FREE),
        })
    res = run_bass_kernel_spmd(nc, in_maps, list(range(N_CORES)))
    outs = [np.asarray(res.results[c]["out"]) for c in range(N_CORES)]
    return np.concatenate(outs, axis=2).astype(np.float32)



# revision 7
# speedup vs baseline: 2.1585x; 2.1585x over previous
"""Lookahead depthwise convolution on 8 Trainium2 NeuronCores.

out[t, b, f] = sum_{c=0..K-1} x[t+c, b, f] * weight[f, c], zero-padded at the
right edge. x: (2048, 32, 1280) fp32, weight: (1280, 81) fp32.

Strategy: shard the (fully independent) feature dim across 8 cores, 160
features each. Per feature the time conv is a banded Toeplitz matmul: with
128-wide time tiles, out_j = A_f @ x_j + B_f @ x_{j+1} where (as lhsT, i.e.
contraction index m first)
  A_f[m, t] = w[f, m - t]        (0 <= m - t < K)
  B_f[m, t] = w[f, m + 128 - t]  (0 <= m + 128 - t < K)

v2 vs v1 (379 us):
 - x is cast to fp16 on the host and shipped pre-transposed per core as
   (half, s, f, b) with f split in two halves of 80 -> input DMA halves and
   the on-chip fp32->fp16 cast disappears.
 - output is produced in fp16 in the same (half, s, f, b) layout (host
   transposes back and upcasts) -> output DMA halves and the PSUM eviction
   copy becomes stride-1 in its innermost dim.
 - matmuls cover a 4-block window in the free dim (N=128/96/32 instead of
   16x N=32) so each LDWEIGHTS is amortized over ~4x more streaming cycles.
 - PSUM eviction alternates between the vector and scalar engines.
"""

import numpy as np

import concourse.bass as bass
import concourse.bacc as bacc
import concourse.mybir as mybir
from concourse import tile
from concourse.bass_utils import run_bass_kernel_spmd

S, B, F, K = 2048, 32, 1280, 81
N_CORES = 8
FC = F // N_CORES          # features per core (160)
FH = FC // 2               # features per half-pass (80)
W = 4                      # time blocks (of 128) per matmul window
NW = S // (128 * W)        # windows (4)
CH = FH * B                # free elems per row chunk (2560)
G = 4                      # features per PSUM bank group
NG = FH // G               # psum groups per window (20)

_compiled = None


def _build_program():
    nc = bacc.Bacc("TRN2", target_bir_lowering=False, debug=False)
    f32, f16 = mybir.dt.float32, mybir.dt.float16

    x_in = nc.declare_dram_parameter("x", [2, S, CH], f16, isOutput=False)
    bands_in = nc.declare_dram_parameter("bands", [128, FC * 256], f16,
                                         isOutput=False)
    out_ext = nc.declare_dram_parameter("out", [2, S, CH], f16, isOutput=True)

    # (half, s, c) -> (half, window, partition, block j, c) with s =
    # (w*W + j)*128 + p
    x_r = x_in.rearrange("h (w j p) c -> h w p j c", j=W, p=128)
    out_r = out_ext.rearrange("h (w j p) c -> h w p j c", j=W, p=128)

    with tile.TileContext(nc) as tc:
        with (
            tc.tile_pool(name="bands", bufs=1) as bpool,
            tc.tile_pool(name="x", bufs=3) as xpool,
            tc.tile_pool(name="stage", bufs=2) as spool,
            tc.tile_pool(name="psum", bufs=8, space="PSUM") as ppool,
        ):
            band_tile = bpool.tile([128, FC * 256], f16)
            nc.gpsimd.dma_start(out=band_tile[:], in_=bands_in[:])
            # zero rhs used to close the zero-padded final block's psum
            # columns (a second start=True would clear the whole bank)
            zero_rhs = bpool.tile([128, B], f16)
            nc.vector.memset(zero_rhs[:], 0.0)

            def load_window(h, w):
                xt = xpool.tile([128, W * CH], f16)
                nc.gpsimd.dma_start(
                    out=xt.rearrange("p (j c) -> p j c", j=W),
                    in_=x_r[h, w])
                return xt

            x_cur = load_window(0, 0)
            for h in range(2):
                for w in range(NW):
                    last = w == NW - 1
                    if not last:
                        x_nxt = load_window(h, w + 1)
                    elif h == 0:
                        x_nxt = load_window(1, 0)
                    else:
                        x_nxt = None
                    # views: free dims (j, f, b)
                    xv = x_cur.rearrange("p (j f b) -> p j f b", j=W, b=B)
                    nv = (x_nxt.rearrange("p (j f b) -> p j f b", j=W, b=B)
                          if x_nxt is not None else None)
                    stage = spool.tile([128, W * CH], f16)
                    for g in range(NG):
                        psum = ppool.tile([128, G * W * B], f32)
                        for f4 in range(G):
                            fh = g * G + f4
                            base = (h * FH + fh) * 256
                            lA = band_tile[:, base:base + 128]
                            lB = band_tile[:, base + 128:base + 256]
                            pc = psum[:, f4 * 128:(f4 + 1) * 128]
                            nc.tensor.matmul(
                                out=pc[:, 0:128], lhsT=lA,
                                rhs=xv[:, :, fh, :],
                                start=True, stop=False)
                            nc.tensor.matmul(
                                out=pc[:, 0:96], lhsT=lB,
                                rhs=xv[:, 1:4, fh, :],
                                start=False, stop=True)
                            # the final block's lookahead is zero-padded
                            nc.tensor.matmul(
                                out=pc[:, 96:128], lhsT=lB,
                                rhs=(nv[:, 0, fh, :] if not last
                                     else zero_rhs[:]),
                                start=False, stop=True)
                        # psum free layout (f4, j, b) -> stage (j, f, b)
                        pv = psum.rearrange("p (f j b) -> p j f b", f=G, j=W)
                        sv = stage.rearrange("p (j f b) -> p j f b", j=W, b=B)
                        eng = nc.vector.tensor_copy if g % 2 == 0 \
                            else nc.scalar.copy
                        eng(out=sv[:, :, g * G:(g + 1) * G, :], in_=pv)
                    nc.gpsimd.dma_start(
                        out=out_r[h, w],
                        in_=stage.rearrange("p (j c) -> p j c", j=W))
                    x_cur = x_nxt
    nc.finalize()
    return nc


def _build_bands(weight):
    # lhsT layout: bands[m, f, 0, t] = w[f, m-t], bands[m, f, 1, t] =
    # w[f, m+128-t]; contraction index m is the partition dim.
    m = np.arange(128)[:, None]
    t = np.arange(128)[None, :]
    dA = m - t
    dB = m + 128 - t
    mA = (dA >= 0) & (dA < K)
    mB = (dB >= 0) & (dB < K)
    iA = np.clip(dA, 0, K - 1)
    iB = np.clip(dB, 0, K - 1)
    w16 = weight.astype(np.float16).astype(np.float32)
    A = w16[:, iA] * mA          # [F, 128m, 128t]
    Bm = w16[:, iB] * mB
    bands = np.empty((128, F, 2, 128), np.float16)
    bands[:, :, 0, :] = A.transpose(1, 0, 2)
    bands[:, :, 1, :] = Bm.transpose(1, 0, 2)
    return bands


def _prep_inputs(x, weight):
    """Per-core input maps: x as fp16 (half, s, f, b); bands fp16."""
    x16 = np.ascontiguousarray(x, dtype=np.float16)
    bands = _build_bands(np.asarray(weight, dtype=np.float32))
    in_maps = []
    for c in range(N_CORES):
        fl = slice(c * FC, (c + 1) * FC)
        xc = x16[:, :, fl].reshape(S, B, 2, FH).transpose(2, 0, 3, 1)
        in_maps.append({
            "x": np.ascontiguousarray(xc).reshape(2, S, CH),
            "bands": np.ascontiguousarray(
                bands[:, fl, :, :]).reshape(128, FC * 256),
        })
    return in_maps


def _post_outputs(res):
    outs = []
    for c in range(N_CORES):
        o = np.asarray(res.results[c]["out"]).reshape(2, S, FH, B)
        outs.append(o.transpose(1, 3, 0, 2).reshape(S, B, FC))
    return np.concatenate(outs, axis=2).astype(np.float32)


def kernel(x, weight):
    global _compiled
    if _compiled is None:
        _compiled = _build_program()
    in_maps = _prep_inputs(x, weight)
    res = run_bass_kernel_spmd(_compiled, in_maps, list(range(N_CORES)))
    return _post_outputs(res)
